# revision 27
# baseline (speedup 1.0000x reference)
"""Trainium2 Bass kernel for nn_BDHAttention (RoPE(Q) self-score attention, no softmax).

Per (batch, head) slice s: QR = rope(Q_s) [T,N]; S = QR @ QR.T / sqrt(N) [T,T];
O_s = S @ V_s [T,N].  K input is unused by the reference.  B*nh = 8 slices map
1:1 onto the 8 NeuronCores (data/head parallel, no communication).

Device-side structure per core (T=2048, N=4096, P=128):
  - Q arrives fp16 with its feature dim de-interleaved on the host
    ([evens | odds]) so RoPE is all contiguous 16-bit tensor_tensor ops
    (DVE 2x mode).  The n-permutation is harmless: it is the contraction
    dim of S = QR @ QR.T and both operands share it.
  - cos/sin tables are host-precomputed fp16, scaled by 1/8 each so S picks
    up the 1/64 = 1/sqrt(N) scale for free.
  - PE-transpose QR' 128x128 tiles into two resident fp16 panels
    (QR'^T, t-halves A and B).  Panel-B build is interleaved with the
    S[A,A] matmuls to keep the PE dense (HAM stays warm; junk identity
    matmuls fill the DVE-bound build windows).
  - MM1 (fp16, fp32 PSUM accum): S[A,A], S[B,B], S[A,B] all from resident
    panels.  Within the diagonal quadrants only on/above-diagonal 128-blocks
    are computed; below-diagonal blocks (and all of S[B,A]) are filled by
    PE-transposing the computed mirrors (S is symmetric).  S stored fp16 in
    a DRAM scratch.
  - MM2: O = S @ V.  S row-panels re-read from DRAM serve directly as lhsT
    tiles (partition = contraction dim) thanks to S's symmetry; V streamed
    fp16; O accumulated fp32 in PSUM and written out fp32.
"""

import math
import sys

sys.path.insert(0, "/opt/trn_rl_repo")

import numpy as np

import concourse.bacc as bacc
import concourse.mybir as mybir
import concourse.tile as tile
from concourse.bass_utils import run_bass_kernel_spmd

B, NH, T, N = 2, 4, 2048, 4096
THETA = 2 ** 16
P = 128
HALF = T // 2            # 1024
NTILES = T // P          # 16 t-tiles
NCH = N // P             # 32 n-chunks
F = 512                  # matmul moving free dim (one fp32 PSUM bank)
H = N // 2               # 2048

f16 = mybir.dt.float16
f32 = mybir.dt.float32


def _build_nc():
    nc = bacc.Bacc("TRN2", target_bir_lowering=False, debug=False, num_devices=8)

    q = nc.dram_tensor("q", [T, N], f16, kind="ExternalInput")
    v = nc.dram_tensor("v", [T, N], f16, kind="ExternalInput")
    cu = nc.dram_tensor("cu", [T, H], f16, kind="ExternalInput")
    su = nc.dram_tensor("su", [T, H], f16, kind="ExternalInput")
    ident = nc.dram_tensor("ident", [P, P], f16, kind="ExternalInput")
    o = nc.dram_tensor("o", [T, N], f32, kind="ExternalOutput")

    with tile.TileContext(nc) as tc:
        with (
            tc.tile_pool(name="dram", bufs=1, space="DRAM") as dram,
            tc.tile_pool(name="const", bufs=1) as const,
            tc.tile_pool(name="panel", bufs=1) as panel,
            tc.tile_pool(name="ps", bufs=1, space="PSUM") as ps,
            tc.tile_pool(name="work", bufs=1) as work,
        ):
            s_mat = dram.tile([T, T], f16, name="s_mat")

            idt = const.tile([P, P], f16, name="idt")
            nc.sync.dma_start(idt[:], ident.ap())

            pa = [
                panel.tile([P, HALF], f16, name=f"pk_a{k}", tag=f"pk_a{k}")
                for k in range(NCH)
            ]
            pb = [
                panel.tile([P, HALF], f16, name=f"pk_b{k}", tag=f"pk_b{k}")
                for k in range(NCH)
            ]

            def build_tile(dst, half, ti):
                """RoPE t-tile (half*8 + ti) and transpose its 32 n-chunks into
                panel columns ti*P:(ti+1)*P."""
                trow = half * (NTILES // 2) + ti
                qt = work.tile([P, N], f16, name="qt", tag="qt", bufs=1)
                cut = work.tile([P, H], f16, name="cut", tag="cut", bufs=1)
                sut = work.tile([P, H], f16, name="sut", tag="sut", bufs=1)
                nc.sync.dma_start(qt[:], q.ap()[trow * P:(trow + 1) * P, :])
                nc.sync.dma_start(cut[:], cu.ap()[trow * P:(trow + 1) * P, :])
                nc.sync.dma_start(sut[:], su.ap()[trow * P:(trow + 1) * P, :])
                qr = work.tile([P, N], f16, name="qr", tag="qr", bufs=1)
                t1 = work.tile([P, H], f16, name="t1", tag="t1", bufs=1)
                t2 = work.tile([P, H], f16, name="t2", tag="t2", bufs=1)
                qe, qo = qt[:, 0:H], qt[:, H:N]
                nc.vector.tensor_mul(t1[:], qe, cut[:])
                nc.vector.tensor_mul(t2[:], qo, sut[:])
                nc.vector.tensor_sub(qr[:, 0:H], t1[:], t2[:])
                nc.vector.tensor_mul(t1[:], qo, cut[:])
                nc.vector.tensor_mul(t2[:], qe, sut[:])
                nc.vector.tensor_add(qr[:, H:N], t1[:], t2[:])
                for k in range(NCH):
                    pt = ps.tile([P, P], f16, name="tr", tag="tr", bufs=2)
                    nc.tensor.transpose(pt[:], qr[:, k * P:(k + 1) * P], idt[:])
                    nc.scalar.copy(dst[k][:, ti * P:(ti + 1) * P], pt[:])

            def s_block(psrc, row, col, width):
                """Evacuate one accumulated S block [P, width] to s_mat rows
                row.., cols col..; returns the fp16 staging tile."""
                st = work.tile([P, width], f16, name="sst", tag="sst", bufs=3)
                nc.vector.tensor_copy(st[:], psrc[:])
                nc.sync.dma_start(s_mat[row:row + P, col:col + width], st[:])
                return st

            def quad_group(lhs_panel, rhs_panel, m, c0, width):
                """One S block: rows m*P of lhs half, cols [c0, c0+width) of
                rhs half (element offsets)."""
                acc = ps.tile([P, width], f32, name="acc", tag="acc", bufs=6)
                for k in range(NCH):
                    nc.tensor.matmul(
                        acc[:],
                        lhs_panel[k][:, m * P:(m + 1) * P],
                        rhs_panel[k][:, c0:c0 + width],
                        start=(k == 0),
                        stop=(k == NCH - 1),
                    )
                return acc

            def mirror_one(st, sub, r0, c0):
                """Write the transpose of st's sub-block [P, P] (cols sub*P..)
                to s_mat rows r0.., cols c0.. (symmetric fill)."""
                pt = ps.tile([P, P], f16, name="tr", tag="tr", bufs=2)
                nc.tensor.transpose(pt[:], st[:, sub * P:(sub + 1) * P], idt[:])
                ft = work.tile([P, P], f16, name="ft", tag="ft", bufs=3)
                nc.scalar.copy(ft[:], pt[:])
                nc.sync.dma_start(s_mat[r0:r0 + P, c0:c0 + P], ft[:])

            def diag_quadrant_row(pan, q0, m):
                """Row-chunk m of a diagonal quadrant (origin q0 in s_mat):
                compute only blocks on/above the diagonal; mirror-fill the
                strictly-above blocks into the skipped mirror positions."""
                for fc in range(FH):
                    j0 = max(0, m - 4 * fc)
                    if j0 >= F // P:
                        continue
                    width = (F // P - j0) * P
                    c0 = fc * F + j0 * P
                    acc = quad_group(pan, pan, m, c0, width)
                    st = s_block(acc, q0 + m * P, q0 + c0, width)
                    for sub in range(width // P):
                        c = 4 * fc + j0 + sub
                        if c > m:
                            mirror_one(st, sub, q0 + c * P, q0 + m * P)

            def pe_warm(nmm):
                """Junk matmuls (on the const identity, so no data deps) to
                keep the PE activity monitor at full clock while the pipeline
                is otherwise DVE/DMA-bound."""
                wacc = ps.tile([P, P], f32, name="wacc", tag="tr", bufs=2)
                for i in range(nmm):
                    nc.tensor.matmul(
                        wacc[:], idt[:], idt[:],
                        start=True, stop=True, skip_group_check=True,
                    )

            MH = HALF // P   # 8 m-chunks per half
            FH = HALF // F   # 2 f-cols per half

            # ---- build panel A (junk MMs keep the PE clock warm) ----
            pe_warm(48)
            for ti in range(MH):
                build_tile(pa, 0, ti)
                pe_warm(64)

            # ---- S[A,A] (diag-block skipping) interleaved with panel-B build ----
            for i in range(MH):
                diag_quadrant_row(pa, 0, i)
                pe_warm(12 * max(0, i - 2))
                build_tile(pb, 1, i)

            # ---- S[B,B] (diag-block skipping), S[A,B] (+ mirror to S[B,A]) ----
            pe_warm(16)
            for m in range(MH):
                diag_quadrant_row(pb, HALF, m)
                for fc in range(FH):
                    acc = quad_group(pa, pb, m, fc * F, F)
                    st = s_block(acc, m * P, HALF + fc * F, F)
                    for sub in range(F // P):
                        mirror_one(st, sub, HALF + fc * F + sub * P, m * P)
                pe_warm(16)

            # ---- MM2: O = S @ V (S row-panels as lhsT via symmetry) ----
            vts0 = []
            for k in range(NTILES):
                vt = work.tile([P, F], f16, name=f"vt_{k}", tag=f"vt_{k}", bufs=2)
                nc.sync.dma_start(vt[:], v.ap()[k * P:(k + 1) * P, 0:F])
                vts0.append(vt)

            srow = []
            for k in range(NTILES):
                u = panel.tile([P, HALF], f16, name=f"pk_a{2 * k}", tag=f"pk_a{2 * k}")
                w = panel.tile(
                    [P, HALF], f16, name=f"pk_a{2 * k + 1}", tag=f"pk_a{2 * k + 1}"
                )
                nc.sync.dma_start(u[:], s_mat[k * P:(k + 1) * P, 0:HALF])
                nc.sync.dma_start(w[:], s_mat[k * P:(k + 1) * P, HALF:T])
                srow.append((u, w))

            pe_warm(96)
            for j in range(N // F):
                if j == 0:
                    vts = vts0
                else:
                    vts = []
                    for k in range(NTILES):
                        vt = work.tile(
                            [P, F], f16, name=f"vt_{k}", tag=f"vt_{k}", bufs=2
                        )
                        nc.sync.dma_start(
                            vt[:], v.ap()[k * P:(k + 1) * P, j * F:(j + 1) * F]
                        )
                        vts.append(vt)
                for m in range(NTILES):
                    acc = ps.tile([P, F], f32, name="acc", tag="acc", bufs=6)
                    for k in range(NTILES):
                        u, w = srow[k]
                        lhsT = (
                            u[:, m * P:(m + 1) * P]
                            if m < 8
                            else w[:, (m - 8) * P:(m - 7) * P]
                        )
                        nc.tensor.matmul(
                            acc[:], lhsT, vts[k][:],
                            start=(k == 0), stop=(k == NTILES - 1),
                        )
                    ot = work.tile([P, F], f32, name="ot", tag="ot", bufs=3)
                    nc.scalar.copy(ot[:], acc[:])
                    nc.sync.dma_start(
                        o.ap()[m * P:(m + 1) * P, j * F:(j + 1) * F], ot[:]
                    )

    nc.compile()
    return nc


def _tables():
    idx = np.arange(N, dtype=np.float32)
    qq = np.floor(idx / 2.0) * 2.0
    freqs = (1.0 / THETA ** (qq / N) / (2.0 * math.pi)).astype(np.float32)
    fe = freqs[::2]  # [N/2], pairs share a frequency
    ph = (np.arange(T, dtype=np.float32)[:, None] * fe[None, :]).astype(np.float32)
    ang = (np.mod(ph, 1.0) * np.float32(2.0 * math.pi)).astype(np.float32)
    cu_ = (np.cos(ang.astype(np.float64)) / 8.0).astype(np.float16)
    su_ = (np.sin(ang.astype(np.float64)) / 8.0).astype(np.float16)
    return cu_, su_


_NC_CACHE = {}


def kernel(Q, K, V, _trace=False, _tmpdir=None):
    del K  # unused by the reference computation
    if "nc" not in _NC_CACHE:
        _NC_CACHE["nc"] = _build_nc()
    nc = _NC_CACHE["nc"]

    cu_, su_ = _tables()
    ident = np.eye(P, dtype=np.float16)
    Qf = np.asarray(Q, dtype=np.float32)
    # de-interleave feature dim: [evens | odds], fp16
    Qd = np.concatenate([Qf[..., 0::2], Qf[..., 1::2]], axis=-1).astype(np.float16)
    V16 = np.asarray(V, dtype=np.float16)

    in_maps = []
    for c in range(8):
        b, h = divmod(c, NH)
        in_maps.append({
            "q": np.ascontiguousarray(Qd[b, h]),
            "v": np.ascontiguousarray(V16[b, h]),
            "cu": cu_,
            "su": su_,
            "ident": ident,
        })

    kw = {}
    if _trace:
        kw = dict(trace=True, tmpdir=_tmpdir)
    res = run_bass_kernel_spmd(nc, in_maps, list(range(8)), **kw)

    out = np.empty((B, NH, T, N), dtype=np.float32)
    for c in range(8):
        b, h = divmod(c, NH)
        out[b, h] = res.results[c]["o"]
    if _trace:
        kernel.last_exec_time_ns = res.exec_time_ns
    return out


# revision 28
# speedup vs baseline: 1.0376x; 1.0376x over previous
"""Trainium2 Bass kernel for nn_BDHAttention (RoPE(Q) self-score attention, no softmax).

Per (batch, head) slice s: QR = rope(Q_s) [T,N]; S = QR @ QR.T / sqrt(N) [T,T];
O_s = S @ V_s [T,N].  K input is unused by the reference.  B*nh = 8 slices map
1:1 onto the 8 NeuronCores (data/head parallel, no communication).

Device-side structure per core (T=2048, N=4096, P=128):
  - Q arrives fp16 with its feature dim de-interleaved on the host
    ([evens | odds]) so RoPE is all contiguous 16-bit tensor_tensor ops
    (DVE 2x mode).  The n-permutation is harmless: it is the contraction
    dim of S = QR @ QR.T and both operands share it.
  - cos/sin tables are host-precomputed fp16, scaled by 1/8 each so S picks
    up the 1/64 = 1/sqrt(N) scale for free.
  - PE-transpose QR' 128x128 tiles into two resident fp16 panels
    (QR'^T, t-halves A and B).  Panel-B build is interleaved with the
    S[A,A] matmuls to keep the PE dense (HAM stays warm; junk identity
    matmuls fill the DVE-bound build windows).
  - MM1 (fp16, fp32 PSUM accum): S[A,A], S[B,B], S[A,B] all from resident
    panels.  Within the diagonal quadrants only on/above-diagonal 128-blocks
    are computed; below-diagonal blocks (and all of S[B,A]) are filled by
    PE-transposing the computed mirrors (S is symmetric).  S stored fp16 in
    a DRAM scratch.
  - MM2: O = S @ V.  S row-panels re-read from DRAM serve directly as lhsT
    tiles (partition = contraction dim) thanks to S's symmetry; V streamed
    fp16; O accumulated fp32 in PSUM and written out fp32.
"""

import math
import sys

sys.path.insert(0, "/opt/trn_rl_repo")

import numpy as np

import concourse.bacc as bacc
import concourse.mybir as mybir
import concourse.tile as tile
from concourse.bass_utils import run_bass_kernel_spmd

B, NH, T, N = 2, 4, 2048, 4096
THETA = 2 ** 16
P = 128
HALF = T // 2            # 1024
NTILES = T // P          # 16 t-tiles
NCH = N // P             # 32 n-chunks
F = 512                  # matmul moving free dim (one fp32 PSUM bank)
H = N // 2               # 2048

f16 = mybir.dt.float16
f32 = mybir.dt.float32


def _build_nc():
    nc = bacc.Bacc("TRN2", target_bir_lowering=False, debug=False, num_devices=8)

    q = nc.dram_tensor("q", [T, N], f16, kind="ExternalInput")
    v = nc.dram_tensor("v", [T, N], f16, kind="ExternalInput")
    cu = nc.dram_tensor("cu", [T, H], f16, kind="ExternalInput")
    su = nc.dram_tensor("su", [T, H], f16, kind="ExternalInput")
    ident = nc.dram_tensor("ident", [P, P], f16, kind="ExternalInput")
    o = nc.dram_tensor("o", [T, N], f32, kind="ExternalOutput")

    with tile.TileContext(nc) as tc:
        with (
            tc.tile_pool(name="dram", bufs=1, space="DRAM") as dram,
            tc.tile_pool(name="const", bufs=1) as const,
            tc.tile_pool(name="panel", bufs=1) as panel,
            tc.tile_pool(name="ps", bufs=1, space="PSUM") as ps,
            tc.tile_pool(name="work", bufs=1) as work,
        ):
            s_mat = dram.tile([T, T], f16, name="s_mat")

            idt = const.tile([P, P], f16, name="idt")
            nc.sync.dma_start(idt[:], ident.ap())

            pa = [
                panel.tile([P, HALF], f16, name=f"pk_a{k}", tag=f"pk_a{k}")
                for k in range(NCH)
            ]
            pb = [
                panel.tile([P, HALF], f16, name=f"pk_b{k}", tag=f"pk_b{k}")
                for k in range(NCH)
            ]

            def build_tile(dst, half, ti):
                """RoPE t-tile (half*8 + ti) and transpose its 32 n-chunks into
                panel columns ti*P:(ti+1)*P."""
                trow = half * (NTILES // 2) + ti
                qt = work.tile([P, N], f16, name="qt", tag="qt", bufs=1)
                cut = work.tile([P, H], f16, name="cut", tag="cut", bufs=1)
                sut = work.tile([P, H], f16, name="sut", tag="sut", bufs=1)
                nc.sync.dma_start(qt[:], q.ap()[trow * P:(trow + 1) * P, :])
                nc.sync.dma_start(cut[:], cu.ap()[trow * P:(trow + 1) * P, :])
                nc.sync.dma_start(sut[:], su.ap()[trow * P:(trow + 1) * P, :])
                qr = work.tile([P, N], f16, name="qr", tag="qr", bufs=1)
                t1 = work.tile([P, H], f16, name="t1", tag="t1", bufs=1)
                t2 = work.tile([P, H], f16, name="t2", tag="t2", bufs=1)
                qe, qo = qt[:, 0:H], qt[:, H:N]
                nc.vector.tensor_mul(t1[:], qe, cut[:])
                nc.vector.tensor_mul(t2[:], qo, sut[:])
                nc.vector.tensor_sub(qr[:, 0:H], t1[:], t2[:])
                nc.vector.tensor_mul(t1[:], qo, cut[:])
                nc.vector.tensor_mul(t2[:], qe, sut[:])
                nc.vector.tensor_add(qr[:, H:N], t1[:], t2[:])
                for k in range(NCH):
                    pt = ps.tile([P, P], f16, name="tr", tag="tr", bufs=2)
                    nc.tensor.transpose(pt[:], qr[:, k * P:(k + 1) * P], idt[:])
                    nc.scalar.copy(dst[k][:, ti * P:(ti + 1) * P], pt[:])

            def s_block(psrc, row, col, width):
                """Evacuate one accumulated S block [P, width] to s_mat rows
                row.., cols col..; returns the fp16 staging tile."""
                st = work.tile([P, width], f16, name="sst", tag="sst", bufs=3)
                nc.vector.tensor_copy(st[:], psrc[:])
                nc.sync.dma_start(s_mat[row:row + P, col:col + width], st[:])
                return st

            def quad_group(lhs_panel, rhs_panel, m, c0, width):
                """One S block: rows m*P of lhs half, cols [c0, c0+width) of
                rhs half (element offsets)."""
                acc = ps.tile([P, width], f32, name="acc", tag="acc", bufs=6)
                for k in range(NCH):
                    nc.tensor.matmul(
                        acc[:],
                        lhs_panel[k][:, m * P:(m + 1) * P],
                        rhs_panel[k][:, c0:c0 + width],
                        start=(k == 0),
                        stop=(k == NCH - 1),
                    )
                return acc

            def mirror_one(st, sub, r0, c0):
                """Write the transpose of st's sub-block [P, P] (cols sub*P..)
                to s_mat rows r0.., cols c0.. (symmetric fill)."""
                pt = ps.tile([P, P], f16, name="tr", tag="tr", bufs=2)
                nc.tensor.transpose(pt[:], st[:, sub * P:(sub + 1) * P], idt[:])
                ft = work.tile([P, P], f16, name="ft", tag="ft", bufs=3)
                nc.scalar.copy(ft[:], pt[:])
                nc.sync.dma_start(s_mat[r0:r0 + P, c0:c0 + P], ft[:])

            def diag_quadrant_row(pan, q0, m):
                """Row-chunk m of a diagonal quadrant (origin q0 in s_mat):
                compute only blocks on/above the diagonal; mirror-fill the
                strictly-above blocks into the skipped mirror positions."""
                for fc in range(FH):
                    j0 = max(0, m - 4 * fc)
                    if j0 >= F // P:
                        continue
                    width = (F // P - j0) * P
                    c0 = fc * F + j0 * P
                    acc = quad_group(pan, pan, m, c0, width)
                    st = s_block(acc, q0 + m * P, q0 + c0, width)
                    for sub in range(width // P):
                        c = 4 * fc + j0 + sub
                        if c > m:
                            mirror_one(st, sub, q0 + c * P, q0 + m * P)

            def pe_warm(nmm):
                """Junk matmuls (on the const identity, so no data deps) to
                keep the PE activity monitor at full clock while the pipeline
                is otherwise DVE/DMA-bound."""
                wacc = ps.tile([P, P], f32, name="wacc", tag="tr", bufs=2)
                for i in range(nmm):
                    nc.tensor.matmul(
                        wacc[:], idt[:], idt[:],
                        start=True, stop=True, skip_group_check=True,
                    )

            MH = HALF // P   # 8 m-chunks per half
            FH = HALF // F   # 2 f-cols per half

            # ---- build panel A (junk MMs keep the PE clock warm) ----
            pe_warm(48)
            for ti in range(MH):
                build_tile(pa, 0, ti)
                pe_warm(32)

            # ---- S[A,A] (diag-block skipping) interleaved with panel-B build ----
            for i in range(MH):
                diag_quadrant_row(pa, 0, i)
                build_tile(pb, 1, i)

            # ---- S[B,B] (diag-block skipping), S[A,B] (+ mirror to S[B,A]) ----
            pe_warm(16)
            for m in range(MH):
                diag_quadrant_row(pb, HALF, m)
                for fc in range(FH):
                    acc = quad_group(pa, pb, m, fc * F, F)
                    st = s_block(acc, m * P, HALF + fc * F, F)
                    for sub in range(F // P):
                        mirror_one(st, sub, HALF + fc * F + sub * P, m * P)

            # ---- MM2: O = S @ V (S row-panels as lhsT via symmetry) ----
            vts0 = []
            for k in range(NTILES):
                vt = work.tile([P, F], f16, name=f"vt_{k}", tag=f"vt_{k}", bufs=2)
                nc.sync.dma_start(vt[:], v.ap()[k * P:(k + 1) * P, 0:F])
                vts0.append(vt)

            srow = []
            for k in range(NTILES):
                u = panel.tile([P, HALF], f16, name=f"pk_a{2 * k}", tag=f"pk_a{2 * k}")
                w = panel.tile(
                    [P, HALF], f16, name=f"pk_a{2 * k + 1}", tag=f"pk_a{2 * k + 1}"
                )
                nc.sync.dma_start(u[:], s_mat[k * P:(k + 1) * P, 0:HALF])
                nc.sync.dma_start(w[:], s_mat[k * P:(k + 1) * P, HALF:T])
                srow.append((u, w))

            pe_warm(24)
            for j in range(N // F):
                if j == 0:
                    vts = vts0
                else:
                    vts = []
                    for k in range(NTILES):
                        vt = work.tile(
                            [P, F], f16, name=f"vt_{k}", tag=f"vt_{k}", bufs=2
                        )
                        nc.sync.dma_start(
                            vt[:], v.ap()[k * P:(k + 1) * P, j * F:(j + 1) * F]
                        )
                        vts.append(vt)
                for m in range(NTILES):
                    acc = ps.tile([P, F], f32, name="acc", tag="acc", bufs=6)
                    for k in range(NTILES):
                        u, w = srow[k]
                        lhsT = (
                            u[:, m * P:(m + 1) * P]
                            if m < 8
                            else w[:, (m - 8) * P:(m - 7) * P]
                        )
                        nc.tensor.matmul(
                            acc[:], lhsT, vts[k][:],
                            start=(k == 0), stop=(k == NTILES - 1),
                        )
                    ot = work.tile([P, F], f32, name="ot", tag="ot", bufs=3)
                    nc.scalar.copy(ot[:], acc[:])
                    nc.sync.dma_start(
                        o.ap()[m * P:(m + 1) * P, j * F:(j + 1) * F], ot[:]
                    )

    nc.compile()
    return nc


def _tables():
    idx = np.arange(N, dtype=np.float32)
    qq = np.floor(idx / 2.0) * 2.0
    freqs = (1.0 / THETA ** (qq / N) / (2.0 * math.pi)).astype(np.float32)
    fe = freqs[::2]  # [N/2], pairs share a frequency
    ph = (np.arange(T, dtype=np.float32)[:, None] * fe[None, :]).astype(np.float32)
    ang = (np.mod(ph, 1.0) * np.float32(2.0 * math.pi)).astype(np.float32)
    cu_ = (np.cos(ang.astype(np.float64)) / 8.0).astype(np.float16)
    su_ = (np.sin(ang.astype(np.float64)) / 8.0).astype(np.float16)
    return cu_, su_


_NC_CACHE = {}


def kernel(Q, K, V, _trace=False, _tmpdir=None):
    del K  # unused by the reference computation
    if "nc" not in _NC_CACHE:
        _NC_CACHE["nc"] = _build_nc()
    nc = _NC_CACHE["nc"]

    cu_, su_ = _tables()
    ident = np.eye(P, dtype=np.float16)
    Qf = np.asarray(Q, dtype=np.float32)
    # de-interleave feature dim: [evens | odds], fp16
    Qd = np.concatenate([Qf[..., 0::2], Qf[..., 1::2]], axis=-1).astype(np.float16)
    V16 = np.asarray(V, dtype=np.float16)

    in_maps = []
    for c in range(8):
        b, h = divmod(c, NH)
        in_maps.append({
            "q": np.ascontiguousarray(Qd[b, h]),
            "v": np.ascontiguousarray(V16[b, h]),
            "cu": cu_,
            "su": su_,
            "ident": ident,
        })

    kw = {}
    if _trace:
        kw = dict(trace=True, tmpdir=_tmpdir)
    res = run_bass_kernel_spmd(nc, in_maps, list(range(8)), **kw)

    out = np.empty((B, NH, T, N), dtype=np.float32)
    for c in range(8):
        b, h = divmod(c, NH)
        out[b, h] = res.results[c]["o"]
    if _trace:
        kernel.last_exec_time_ns = res.exec_time_ns
    return out
